# revision 12
# baseline (speedup 1.0000x reference)
"""Trainium2 Bass kernel for nn_Logalike_40072044871937.

Computes the Lorentz-hyperboloid CTMC log-likelihood:
    ll = sum_{c != i, s} log( pi * (P[c,s,0,si_s] * P[c,s,0,sj_cs]
                                    + [sj==si!=0] * P[c,s,si_s,si_s]^2) )
with P[c,s] = expm(t_c * Q_s),  t_c = 0.5 * arccosh(<x_i, x_c>_L clamp).

Algorithm: since M = t_c * Q_s is a scalar-scaled matrix, rows of expm(M)
are Taylor series in t_c.  With the positivity shift B = Q + lam*I (lam =
max -diag(Q), so B >= 0 entrywise and the series has no cancellation):

    P[c,s,r,m] = exp(-lam * t_c) * sum_k (t_c^k / k!) * (B_s^k)[r,m]

Per-site row-power tables (B_s^k rows 0 and si_s, scaled by 1/k!) are tiny
(O(S*K*n^2) = 3 MFLOP) and staged host-side; all O(C*S*n) work runs on
device: the Taylor contraction is a [K,64]^T @ [K, S*n] matmul per core,
the sj gather is a one-hot multiply + grouped reduce, and the log + masked
reduction finish on-chip.  Cells (C=512) are sharded 64-per-core across 8
NeuronCores; the exp(-lam t) prefactor folds into log-space as a per-cell
linear term, and the pi=1/n constant is added on host exactly.
"""

import numpy as np

import concourse.bass as bass
import concourse.bacc as bacc
import concourse.tile as tile
import concourse.mybir as mybir
from concourse.bass_utils import run_bass_kernel_spmd

# problem shape (hardcoded per contract)
C, S, N, D = 512, 256, 16, 8
K = 16            # Taylor terms; ||t*B||_inf <= 1.66 -> term 15 < 1e-10
NCORES = 8
CSH = C // NCORES  # 64 cells per core
RHO = 1.0
F32 = mybir.dt.float32

_CACHE = {}


def _build_nc():
    nc = bacc.Bacc("TRN2", target_bir_lowering=False, debug=False)
    xt = nc.declare_dram_parameter("xt", [D + 1, CSH], F32, isOutput=False)
    a9 = nc.declare_dram_parameter("a9", [D + 1, 1], F32, isOutput=False)
    krow = nc.declare_dram_parameter("krow", [1, K], F32, isOutput=False)
    r0 = nc.declare_dram_parameter("r0", [K, S * N], F32, isOutput=False)
    a0 = nc.declare_dram_parameter("a0", [K, S], F32, isOutput=False)
    ai = nc.declare_dram_parameter("ai", [K, S], F32, isOutput=False)
    chf = nc.declare_dram_parameter("chf", [CSH, S], F32, isOutput=False)
    sir = nc.declare_dram_parameter("sir", [CSH, S], F32, isOutput=False)
    mio = nc.declare_dram_parameter("mio", [CSH, N], F32, isOutput=False)
    val = nc.declare_dram_parameter("val", [CSH, 1], F32, isOutput=False)
    vlam = nc.declare_dram_parameter("vlam", [CSH, 1], F32, isOutput=False)
    out = nc.declare_dram_parameter("out", [1, 1], F32, isOutput=True)

    EPS1 = float(np.float32(1.0 + 1e-6))
    AF = mybir.ActivationFunctionType
    ALU = mybir.AluOpType
    NCHUNK = 8
    CW = (S * N) // NCHUNK  # 512 columns per chunk
    SCH = CW // N           # 32 sites per chunk

    with tile.TileContext(nc) as tc:
        with (
            tc.tile_pool(name="consts", bufs=1) as consts,
            tc.tile_pool(name="work", bufs=1) as work,
            tc.tile_pool(name="prod", bufs=3) as prodp,
            tc.tile_pool(name="psmall", bufs=1, space="PSUM") as psmall,
            tc.tile_pool(name="psig", bufs=1, space="PSUM") as psig,
            tc.tile_pool(name="pchunk", bufs=3, space="PSUM") as pchunk,
        ):
            # ---- input DMAs ----
            s_xt = consts.tile([D + 1, CSH], F32)
            nc.sync.dma_start(s_xt[:], xt[:])
            s_a9 = consts.tile([D + 1, 1], F32)
            nc.sync.dma_start(s_a9[:], a9[:])
            s_krow = consts.tile([1, K], F32)
            nc.sync.dma_start(s_krow[:], krow[:])
            s_r0 = consts.tile([K, S * N], F32)
            nc.sync.dma_start(s_r0[:], r0[:])
            s_a0 = consts.tile([K, S], F32)
            nc.sync.dma_start(s_a0[:], a0[:])
            s_ai = consts.tile([K, S], F32)
            nc.sync.dma_start(s_ai[:], ai[:])
            s_chf = consts.tile([CSH, S], F32)
            nc.sync.dma_start(s_chf[:], chf[:])
            s_sir = consts.tile([CSH, S], F32)
            nc.sync.dma_start(s_sir[:], sir[:])
            s_mio = consts.tile([CSH, N], F32)
            nc.sync.dma_start(s_mio[:], mio[:])
            s_val = consts.tile([CSH, 1], F32)
            nc.sync.dma_start(s_val[:], val[:])
            s_vlam = consts.tile([CSH, 1], F32)
            nc.sync.dma_start(s_vlam[:], vlam[:])

            s_neg1 = consts.tile([CSH, 1], F32)
            nc.vector.memset(s_neg1[:], -1.0)

            # ---- t chain, row layout (for W) ----
            # upre = a9 . X  (= -inner/rho);  row [1,64] and col [64,1]
            p_urow = psmall.tile([1, CSH], F32)
            nc.tensor.matmul(p_urow[:], s_a9[:], s_xt[:], start=True, stop=True)
            p_ucol = psmall.tile([CSH, 1], F32)
            nc.tensor.matmul(p_ucol[:], s_xt[:], s_a9[:], start=True, stop=True)

            # u = max(upre, 1+1e-6); acosh(u) = ln(u + sqrt(u^2-1))
            # sqrt via exp(0.5*ln(.)) to stay in the ln/exp table set
            s_urow = work.tile([1, CSH], F32)
            nc.vector.tensor_scalar_max(s_urow[:], p_urow[:], EPS1)
            s_y = work.tile([1, CSH], F32)
            nc.vector.tensor_mul(s_y[:], s_urow[:], s_urow[:])
            s_lny = work.tile([1, CSH], F32)
            nc.scalar.activation(s_lny[:], s_y[:], AF.Ln, bias=s_neg1[:1, :])
            s_sq = work.tile([1, CSH], F32)
            nc.scalar.activation(s_sq[:], s_lny[:], AF.Exp, scale=0.5)
            s_s4 = work.tile([1, CSH], F32)
            nc.vector.tensor_add(s_s4[:], s_urow[:], s_sq[:])
            s_L = work.tile([1, CSH], F32)
            nc.scalar.activation(s_L[:], s_s4[:], AF.Ln)       # dist = 2t
            s_lnt = work.tile([1, CSH], F32)
            nc.scalar.activation(s_lnt[:], s_L[:], AF.Ln, scale=0.5)  # ln t

            # W[k,c] = t_c^k = exp(k * ln t)
            p_klnt = psmall.tile([K, CSH], F32, tag="aux")
            nc.tensor.matmul(p_klnt[:], s_krow[:], s_lnt[:], start=True, stop=True)
            s_w = work.tile([K, CSH], F32)
            nc.scalar.activation(s_w[:], p_klnt[:], AF.Exp)

            # ---- t chain, column layout (for the -2*lam*t fold) ----
            s_ucol = work.tile([CSH, 1], F32)
            nc.vector.tensor_scalar_max(s_ucol[:], p_ucol[:], EPS1)
            s_y2 = work.tile([CSH, 1], F32)
            nc.vector.tensor_mul(s_y2[:], s_ucol[:], s_ucol[:])
            s_lny2 = work.tile([CSH, 1], F32)
            nc.scalar.activation(s_lny2[:], s_y2[:], AF.Ln, bias=s_neg1[:])
            s_sq2 = work.tile([CSH, 1], F32)
            nc.scalar.activation(s_sq2[:], s_lny2[:], AF.Exp, scale=0.5)
            s_s4c = work.tile([CSH, 1], F32)
            nc.vector.tensor_add(s_s4c[:], s_ucol[:], s_sq2[:])
            s_lc = work.tile([CSH, 1], F32)
            nc.scalar.activation(s_lc[:], s_s4c[:], AF.Ln)     # dist_c = 2 t_c

            # ---- sigma matmuls ----
            p_sig0si = psig.tile([CSH, S], F32)
            nc.tensor.matmul(p_sig0si[:], s_w[:], s_a0[:], start=True, stop=True)
            p_sigssi = psig.tile([CSH, S], F32)
            nc.tensor.matmul(p_sigssi[:], s_w[:], s_ai[:], start=True, stop=True)

            # ---- one-hot of char over the 16 states ----
            s_onehot = work.tile([CSH, S, N], F32)
            nc.vector.tensor_tensor(
                out=s_onehot[:],
                in0=s_chf[:, :, None].broadcast_to([CSH, S, N]),
                in1=s_mio[:, None, :].broadcast_to([CSH, S, N]),
                op=ALU.is_equal,
            )

            # ---- P0 chunks: matmul -> mask -> grouped reduce ----
            s_sig0sj = work.tile([CSH, S], F32)
            for j in range(NCHUNK):
                p_ch = pchunk.tile([CSH, CW], F32)
                nc.tensor.matmul(
                    p_ch[:], s_w[:], s_r0[:, j * CW:(j + 1) * CW],
                    start=True, stop=True,
                )
                s_prod = prodp.tile([CSH, SCH, N], F32)
                nc.vector.tensor_tensor(
                    out=s_prod[:],
                    in0=p_ch[:].rearrange("p (s n) -> p s n", n=N),
                    in1=s_onehot[:, j * SCH:(j + 1) * SCH, :],
                    op=ALU.mult,
                )
                nc.vector.tensor_reduce(
                    out=s_sig0sj[:, j * SCH:(j + 1) * SCH],
                    in_=s_prod[:],
                    axis=mybir.AxisListType.X,
                    op=ALU.add,
                )

            # ---- combine: comb = sig0si*sig0sj + same * sigssi^2 ----
            s_same = work.tile([CSH, S], F32)
            nc.vector.tensor_tensor(
                out=s_same[:], in0=s_chf[:], in1=s_sir[:], op=ALU.is_equal
            )
            # (sigssi * same)^2 == sigssi^2 * same  (mask is 0/1);
            # ordering keeps each DVE op to a single PSUM operand
            s_ssm = work.tile([CSH, S], F32)
            nc.vector.tensor_tensor(
                out=s_ssm[:], in0=p_sigssi[:], in1=s_same[:], op=ALU.mult
            )
            s_ss2m = work.tile([CSH, S], F32)
            nc.vector.tensor_tensor(
                out=s_ss2m[:], in0=s_ssm[:], in1=s_ssm[:], op=ALU.mult
            )
            s_p0 = work.tile([CSH, S], F32)
            nc.vector.tensor_tensor(
                out=s_p0[:], in0=p_sig0si[:], in1=s_sig0sj[:], op=ALU.mult
            )
            s_comb = work.tile([CSH, S], F32)
            nc.vector.tensor_tensor(
                out=s_comb[:], in0=s_p0[:], in1=s_ss2m[:], op=ALU.add
            )

            # ---- ln + fused row-sum; fold -S*lam*dist_c; mask & reduce ----
            s_lncomb = work.tile([CSH, S], F32)
            s_acc = work.tile([CSH, 1], F32)
            nc.scalar.activation(
                s_lncomb[:], s_comb[:], AF.Ln, accum_out=s_acc[:]
            )
            s_final = work.tile([CSH, 1], F32)
            # final_c = dist_c * (-S*lam/2 scaling provided in vlam) + acc_c
            nc.vector.scalar_tensor_tensor(
                out=s_final[:], in0=s_lc[:], scalar=s_vlam[:], in1=s_acc[:],
                op0=ALU.mult, op1=ALU.add,
            )
            p_out = psmall.tile([1, 1], F32, tag="aux")
            nc.tensor.matmul(p_out[:], s_final[:], s_val[:], start=True, stop=True)
            s_out = work.tile([1, 1], F32)
            nc.vector.tensor_copy(s_out[:], p_out[:])
            nc.sync.dma_start(out[:], s_out[:])

    nc.finalize()
    return nc


def _host_prep(X, Q, char, i):
    """Build per-core input maps (sharding + tiny O(S*K*n^2) table staging)."""
    X = np.asarray(X, np.float32)
    Q = np.asarray(Q, np.float32)
    char = np.asarray(char, np.int32)
    i = int(np.asarray(i))

    xi = X[i]
    a9 = np.empty((D + 1, 1), np.float32)
    a9[0, 0] = xi[0] / RHO
    a9[1:, 0] = -xi[1:] / RHO

    lam = float(np.max(-np.diagonal(Q, axis1=-2, axis2=-1)).astype(np.float64))
    Bd = Q.astype(np.float64) + lam * np.eye(N)
    si = char[i]  # [S]

    # tables: R0[k, s*N+m] = (B_s^k)[0,m]/k!,  and the si-gathered columns
    R0 = np.zeros((K, S, N), np.float64)
    Ri_si = np.zeros((K, S), np.float64)     # (B_s^k)[si,si]/k!
    r0 = np.zeros((S, N)); r0[:, 0] = 1.0
    ri = np.zeros((S, N)); ri[np.arange(S), si] = 1.0
    fact = 1.0
    for k in range(K):
        if k > 0:
            fact *= k
            r0 = np.einsum('sp,spm->sm', r0, Bd)
            ri = np.einsum('sp,spm->sm', ri, Bd)
        R0[k] = r0 / fact
        Ri_si[k] = ri[np.arange(S), si] / fact
    R0f = np.ascontiguousarray(R0.reshape(K, S * N).astype(np.float32))
    A0 = np.ascontiguousarray(R0.reshape(K, S, N)[:, np.arange(S), si].astype(np.float32))
    Ai = Ri_si.astype(np.float32)
    Ai[:, si == 0] = 0.0                     # ancestor a=s needs s != 0
    Ai = np.ascontiguousarray(Ai)

    krow = np.arange(K, dtype=np.float32).reshape(1, K)
    sir = np.ascontiguousarray(
        np.broadcast_to(si.astype(np.float32)[None, :], (CSH, S)))
    mio = np.ascontiguousarray(
        np.broadcast_to(np.arange(N, dtype=np.float32)[None, :], (CSH, N)))
    # per-(c,s) prefactor term is -2*lam*t_c = -lam*dist_c; summed over S
    # sites -> final_c = dist_c * (-S*lam) + sum_s ln(sigcomb)
    vlam = np.full((CSH, 1), -S * lam, np.float32)

    in_maps = []
    for core in range(NCORES):
        lo = core * CSH
        sl = slice(lo, lo + CSH)
        valid = (np.arange(lo, lo + CSH) != i).astype(np.float32).reshape(CSH, 1)
        in_maps.append({
            "xt": np.ascontiguousarray(X[sl].T),
            "a9": a9,
            "krow": krow,
            "r0": R0f,
            "a0": A0,
            "ai": Ai,
            "chf": np.ascontiguousarray(char[sl].astype(np.float32)),
            "sir": sir,
            "mio": mio,
            "val": valid,
            "vlam": vlam,
        })
    n_valid = C - (1 if 0 <= i < C else 0)
    host_const = float(n_valid) * float(S) * float(np.log(1.0 / N))
    return in_maps, host_const


def run(X, Q, char, i, trace=False):
    if "nc" not in _CACHE:
        _CACHE["nc"] = _build_nc()
    nc = _CACHE["nc"]
    in_maps, host_const = _host_prep(X, Q, char, i)
    res = run_bass_kernel_spmd(nc, in_maps, core_ids=list(range(NCORES)),
                               trace=trace)
    total = host_const + sum(float(r["out"][0, 0]) for r in res.results)
    return np.float32(total), res


def kernel(X, Q, char, i):
    out, _ = run(X, Q, char, i)
    return out
